# revision 1
# baseline (speedup 1.0000x reference)
"""CocoNODE forward on 8 Trainium2 NeuronCores.

Sharding: token-parallel. Rank r owns 256 tokens: batch r//4, positions
[256*(r%4), 256*(r%4)+256). z kept feature-major (z^T) resident in SBUF f32.
Per dynamics eval: local qkv (all heads, own tokens), AllGather of K/V within
batch groups [[0..3],[4..7]], full-square masked attention (mask data makes the
program rank-uniform), local attn_proj/MLP. Final: ln_f, all-8 AllGather of
z_f, vocab-sharded (50304/8=6288) tied lm_head. Matmuls bf16, PSUM f32,
residual/LN stats f32.
"""
import os, sys
sys.path.insert(0, '/opt/trn_rl_repo')

import math
import numpy as np
import ml_dtypes
from contextlib import ExitStack

import concourse.bass as bass
import concourse.tile as tile
from concourse import bacc, mybir
from concourse import tile_cfg
from concourse.bass_utils import run_bass_kernel_spmd

AF = mybir.ActivationFunctionType
OP = mybir.AluOpType
BF = mybir.dt.bfloat16
F32 = mybir.dt.float32

B, T, D, NH, TE, V, NBLK, NSTEPS, HSTEP = 2, 1024, 768, 12, 64, 50304, 4, 4, 0.25
HD = D // NH
EPS = 1e-5
R = 8              # ranks
TOK = 256          # tokens per rank
DKT = 6            # 768/128 k-tiles
XKT = 7            # 896/128 k-tiles (832 ext padded)
VS = V // R        # 6288 vocab shard
NEV = NBLK * NSTEPS
KHALF = TOK * (D + NH * (HD + 1))          # unused marker
AGSZ = 64 * 12 * 256 + 128 * 2 * 780       # K part + V part elems per rank
KOFF = 64 * 12 * 256                        # V region offset in ag block

_CACHED = {}


def build_nc():
    nc = bacc.Bacc("TRN2", target_bir_lowering=False, debug=False, num_devices=R)

    # ---- inputs (per-core DRAM) ----
    z0 = nc.dram_tensor("z0", [128, DKT, TOK], F32, kind="ExternalInput")
    wattn = nc.dram_tensor("wattn", [NBLK, 128, XKT, 3 * D], BF, kind="ExternalInput")
    wap = nc.dram_tensor("wap", [NBLK, 128, DKT, D], BF, kind="ExternalInput")
    apb = nc.dram_tensor("apb", [NBLK, 128, DKT], F32, kind="ExternalInput")
    wfc = nc.dram_tensor("wfc", [NBLK, 128, XKT, 4 * D], BF, kind="ExternalInput")
    wmp = nc.dram_tensor("wmp", [NBLK, 128, 4 * DKT, D], BF, kind="ExternalInput")
    mpb = nc.dram_tensor("mpb", [NBLK, 128, DKT], F32, kind="ExternalInput")
    ln1w = nc.dram_tensor("ln1w", [NBLK, 128, DKT], F32, kind="ExternalInput")
    ln1b = nc.dram_tensor("ln1b", [NBLK, 128, DKT], F32, kind="ExternalInput")
    ln2w = nc.dram_tensor("ln2w", [NBLK, 128, DKT], F32, kind="ExternalInput")
    ln2b = nc.dram_tensor("ln2b", [NBLK, 128, DKT], F32, kind="ExternalInput")
    ln1tw = nc.dram_tensor("ln1tw", [64, NBLK], F32, kind="ExternalInput")
    ln1tb = nc.dram_tensor("ln1tb", [64, NBLK], F32, kind="ExternalInput")
    ln2tw = nc.dram_tensor("ln2tw", [64, NBLK], F32, kind="ExternalInput")
    ln2tb = nc.dram_tensor("ln2tb", [64, NBLK], F32, kind="ExternalInput")
    lnfw = nc.dram_tensor("lnfw", [128, DKT], F32, kind="ExternalInput")
    lnfb = nc.dram_tensor("lnfb", [128, DKT], F32, kind="ExternalInput")
    tembs = nc.dram_tensor("tembs", [64, NEV], F32, kind="ExternalInput")
    tcon = nc.dram_tensor("tcon", [1, 2 * NEV], F32, kind="ExternalInput")
    masks = nc.dram_tensor("masks", [128, 16, TOK], BF, kind="ExternalInput")
    wteT = nc.dram_tensor("wteT", [128, DKT, VS], BF, kind="ExternalInput")
    logits = nc.dram_tensor("logits", [R * TOK, VS], F32, kind="ExternalOutput")
    KDEBUG = os.environ.get("KDEBUG", "0") == "1"
    if KDEBUG:
        dbg_z = nc.dram_tensor("dbg_z", [128, DKT, TOK], F32, kind="ExternalOutput")
        dbg_x = nc.dram_tensor("dbg_x", [128, XKT, TOK], F32, kind="ExternalOutput")
        dbg_q = nc.dram_tensor("dbg_q", [64, NH, TOK], F32, kind="ExternalOutput")
        dbg_y = nc.dram_tensor("dbg_y", [128, DKT, TOK], F32, kind="ExternalOutput")
        dbg_h = nc.dram_tensor("dbg_h", [128, DKT, TOK], F32, kind="ExternalOutput")

    # ---- internal DRAM (collectives) ----
    ag_in = nc.dram_tensor("ag_in", [AGSZ], BF)
    ag_out = nc.dram_tensor("ag_out", [8 * AGSZ], BF)
    agf_in = nc.dram_tensor("agf_in", [128 * DKT * TOK], BF)
    agf_out = nc.dram_tensor("agf_out", [R * 128 * DKT * TOK], BF)
    G4 = [[0, 1, 2, 3], [4, 5, 6, 7]]
    G8 = [[0, 1, 2, 3, 4, 5, 6, 7]]

    with ExitStack() as ctx:
        tc = ctx.enter_context(tile.TileContext(nc))
        # persistent single-slot pools
        pers = ctx.enter_context(tc.tile_pool(name="pers", bufs=1))
        wpool = ctx.enter_context(tc.tile_pool(name="w", bufs=1))
        work = ctx.enter_context(tc.tile_pool(name="work", bufs=2))
        small = ctx.enter_context(tc.tile_pool(name="small", bufs=4))
        kvp = ctx.enter_context(tc.tile_pool(name="kv", bufs=2))
        hfcp = ctx.enter_context(tc.tile_pool(name="hfc", bufs=1))
        psA = ctx.enter_context(tc.tile_pool(name="psA", bufs=2, space="PSUM"))
        psS = ctx.enter_context(tc.tile_pool(name="psS", bufs=2, space="PSUM"))
        psY = ctx.enter_context(tc.tile_pool(name="psY", bufs=4, space="PSUM"))

        z = pers.tile([128, DKT, TOK], F32, tag="z")
        nc.sync.dma_start(out=z, in_=z0[:, :, :])
        msk = pers.tile([128, 16, TOK], BF, tag="msk")
        nc.sync.dma_start(out=msk, in_=masks[:, :, :])
        te_sb = pers.tile([64, NEV], F32, tag="te")
        nc.sync.dma_start(out=te_sb, in_=tembs[:, :])
        tc_sb = pers.tile([1, 2 * NEV], F32, tag="tc")
        nc.sync.dma_start(out=tc_sb, in_=tcon[:, :])
        ltw1 = pers.tile([64, NBLK], F32, tag="ltw1")
        nc.sync.dma_start(out=ltw1, in_=ln1tw[:, :])
        ltb1 = pers.tile([64, NBLK], F32, tag="ltb1")
        nc.sync.dma_start(out=ltb1, in_=ln1tb[:, :])
        ltw2 = pers.tile([64, NBLK], F32, tag="ltw2")
        nc.sync.dma_start(out=ltw2, in_=ln2tw[:, :])
        ltb2 = pers.tile([64, NBLK], F32, tag="ltb2")
        nc.sync.dma_start(out=ltb2, in_=ln2tb[:, :])

        ones_c = pers.tile([128, 1], BF, tag="ones_c")   # stats lhsT [128,1]
        nc.vector.memset(ones_c, 1.0)
        ones_r = pers.tile([1, 128], BF, tag="ones_r")   # bcast lhsT [1,<=128]
        nc.vector.memset(ones_r, 1.0)
        eps_sb = pers.tile([1, 1], F32, tag="eps")
        nc.vector.memset(eps_sb, EPS)

        x_ext = pers.tile([128, XKT, TOK], BF, tag="x_ext")
        nc.vector.memset(x_ext[64:128, XKT - 1, :], 0.0)
        nc.vector.memset(x_ext[64:65, XKT - 1, :], 1.0)

        qt = pers.tile([64, NH, TOK], BF, tag="qt")
        kloc = pers.tile([64, NH, TOK], BF, tag="kloc")
        vloc = pers.tile([128, 2, NH * (HD + 1)], BF, tag="vloc")
        vloc_h = vloc.rearrange("p t (h c) -> p t h c", h=NH)
        nc.vector.memset(vloc_h[:, :, :, HD:HD + 1], 1.0)
        ysb = pers.tile([128, DKT, TOK], BF, tag="ysb")
        hsb = pers.tile([128, DKT, TOK], F32, tag="hsb")
        mu_sb = pers.tile([128, TOK], F32, tag="mu_sb")
        rs_sb = pers.tile([128, TOK], F32, tag="rs_sb")

        wattn_sb = wpool.tile([128, XKT, 3 * D], BF, tag="wattn")
        wap_sb = wpool.tile([128, DKT, D], BF, tag="wap")
        apb_sb = wpool.tile([128, DKT], F32, tag="apb")
        wfc_sb = wpool.tile([128, XKT, 4 * D], BF, tag="wfc")
        wmp_sb = wpool.tile([128, 4 * DKT, D], BF, tag="wmp")
        mpb_sb = wpool.tile([128, DKT], F32, tag="mpb")
        l1w_sb = wpool.tile([128, DKT], F32, tag="l1w")
        l1b_sb = wpool.tile([128, DKT], F32, tag="l1b")
        l2w_sb = wpool.tile([128, DKT], F32, tag="l2w")
        l2b_sb = wpool.tile([128, DKT], F32, tag="l2b")

        agK = ag_in[0:KOFF].rearrange("(p x) -> p x", p=64)          # [64, 3072]
        agV = ag_in[KOFF:AGSZ].rearrange("(p x) -> p x", p=128)      # [128, 1560]

        def ag_blockK(blk):
            o = blk * AGSZ
            return ag_out[o:o + KOFF].rearrange("(p h t) -> p h t", p=64, h=NH)

        def ag_blockV(blk):
            o = blk * AGSZ + KOFF
            return ag_out[o:o + 128 * 1560].rearrange("(p t x) -> p t x", p=128, t=2)

        def layernorm(src_tiles, nkt, divisor, wt, bt, twt, tbt, ev, dst):
            """src_tiles: f32 [128, TOK] slices; writes dst (x_ext) bf16.
            twt/tbt None => ln_f (no t-part, dst kt DKT only)."""
            sums = psS.tile([1, TOK], F32, tag="psS")
            sumsq = psS.tile([1, TOK], F32, tag="psS")
            for k in range(nkt):
                zb = work.tile([128, TOK], BF, tag="zb")
                nc.vector.tensor_copy(out=zb, in_=src_tiles[k])
                zq = work.tile([128, TOK], BF, tag="zq")
                nc.vector.tensor_mul(out=zq, in0=zb, in1=zb)
                nc.tensor.matmul(sums, ones_c, zb, start=(k == 0), stop=(k == nkt - 1))
                nc.tensor.matmul(sumsq, ones_c, zq, start=(k == 0), stop=(k == nkt - 1))
            mu = small.tile([1, TOK], F32, tag="mu")
            if ev is not None:
                nc.vector.tensor_scalar(out=mu, in0=sums, scalar1=tc_sb[0:1, ev:ev + 1],
                                        scalar2=1.0 / divisor, op0=OP.add, op1=OP.mult)
            else:
                nc.vector.tensor_scalar_mul(out=mu, in0=sums, scalar1=1.0 / divisor)
            e2 = small.tile([1, TOK], F32, tag="e2")
            if ev is not None:
                nc.vector.tensor_scalar(out=e2, in0=sumsq, scalar1=tc_sb[0:1, NEV + ev:NEV + ev + 1],
                                        scalar2=1.0 / divisor, op0=OP.add, op1=OP.mult)
            else:
                nc.vector.tensor_scalar_mul(out=e2, in0=sumsq, scalar1=1.0 / divisor)
            mu2 = small.tile([1, TOK], F32, tag="mu2")
            nc.vector.tensor_mul(out=mu2, in0=mu, in1=mu)
            var = small.tile([1, TOK], F32, tag="var")
            nc.vector.tensor_sub(out=var, in0=e2, in1=mu2)
            rst = small.tile([1, TOK], F32, tag="rst")
            nc.scalar.activation(out=rst, in_=var, func=AF.Sqrt, bias=eps_sb, scale=1.0)
            nc.vector.reciprocal(out=rst, in_=rst)
            mu_bf = small.tile([1, TOK], BF, tag="mu_bf")
            nc.vector.tensor_copy(out=mu_bf, in_=mu)
            rs_bf = small.tile([1, TOK], BF, tag="rs_bf")
            nc.vector.tensor_copy(out=rs_bf, in_=rst)
            mu_ps = psS.tile([128, TOK], F32, tag="psS")
            nc.tensor.matmul(mu_ps, ones_r, mu_bf, start=True, stop=True)
            nc.scalar.activation(out=mu_sb, in_=mu_ps, func=AF.Copy)
            rs_ps = psS.tile([128, TOK], F32, tag="psS")
            nc.tensor.matmul(rs_ps, ones_r, rs_bf, start=True, stop=True)
            nc.scalar.activation(out=rs_sb, in_=rs_ps, func=AF.Copy)
            for k in range(nkt):
                t1 = work.tile([128, TOK], F32, tag="t1")
                nc.vector.tensor_sub(out=t1, in0=src_tiles[k], in1=mu_sb)
                nc.vector.tensor_mul(out=t1, in0=t1, in1=rs_sb)
                nc.vector.tensor_scalar(out=dst[:, k, :], in0=t1,
                                        scalar1=wt[:, k:k + 1], scalar2=bt[:, k:k + 1],
                                        op0=OP.mult, op1=OP.add)
            if twt is not None:
                t2 = work.tile([64, TOK], F32, tag="t2")
                nc.vector.tensor_scalar(out=t2, in0=mu_sb[0:64, :], scalar1=-1.0,
                                        scalar2=te_sb[:, ev:ev + 1], op0=OP.mult, op1=OP.add)
                nc.vector.tensor_mul(out=t2, in0=t2, in1=rs_sb[0:64, :])
                nc.vector.tensor_scalar(out=dst[0:64, XKT - 1, :], in0=t2,
                                        scalar1=twt, scalar2=tbt, op0=OP.mult, op1=OP.add)

        NBLK_RUN = 1 if KDEBUG else NBLK
        NSTEPS_RUN = 1 if KDEBUG else NSTEPS
        if os.environ.get("KNULL", "0") == "1":
            NBLK_RUN = 0
        KREPEAT = int(os.environ.get("KREPEAT", "1"))
        for blk in [b % NBLK for b in range(NBLK_RUN * KREPEAT)]:
            nc.sync.dma_start(out=wattn_sb, in_=wattn[blk])
            nc.sync.dma_start(out=wap_sb, in_=wap[blk])
            nc.sync.dma_start(out=apb_sb, in_=apb[blk])
            nc.sync.dma_start(out=wfc_sb, in_=wfc[blk])
            nc.sync.dma_start(out=wmp_sb, in_=wmp[blk])
            nc.sync.dma_start(out=mpb_sb, in_=mpb[blk])
            nc.sync.dma_start(out=l1w_sb, in_=ln1w[blk])
            nc.sync.dma_start(out=l1b_sb, in_=ln1b[blk])
            nc.sync.dma_start(out=l2w_sb, in_=ln2w[blk])
            nc.sync.dma_start(out=l2b_sb, in_=ln2b[blk])
            for st in range(NSTEPS_RUN):
                ev = blk * NSTEPS + st
                # ---- attn_ln ----
                layernorm([z[:, k, :] for k in range(DKT)], DKT, D + TE,
                          l1w_sb, l1b_sb, ltw1[:, blk:blk + 1], ltb1[:, blk:blk + 1],
                          ev, x_ext)
                # ---- qkv: K first (feeds AG), then V, then Q ----
                for m in range(6, 12):
                    ps = psS.tile([128, TOK], F32, tag="psS")
                    for k in range(XKT):
                        nc.tensor.matmul(ps, wattn_sb[:, k, 128 * m:128 * m + 128],
                                         x_ext[:, k, :], start=(k == 0), stop=(k == XKT - 1))
                    hm = 2 * (m - 6)
                    nc.scalar.activation(out=kloc[:, hm, :], in_=ps[0:64, :], func=AF.Copy)
                    nc.scalar.activation(out=kloc[:, hm + 1, :], in_=ps[64:128, :], func=AF.Copy)
                for tt in range(2):
                    ps1 = psA.tile([128, 512], F32, tag="psA")
                    ps2 = psS.tile([128, TOK], F32, tag="psS")
                    for k in range(XKT):
                        nc.tensor.matmul(ps1, x_ext[:, k, 128 * tt:128 * tt + 128],
                                         wattn_sb[:, k, 2 * D:2 * D + 512],
                                         start=(k == 0), stop=(k == XKT - 1))
                    for k in range(XKT):
                        nc.tensor.matmul(ps2, x_ext[:, k, 128 * tt:128 * tt + 128],
                                         wattn_sb[:, k, 2 * D + 512:3 * D],
                                         start=(k == 0), stop=(k == XKT - 1))
                    nc.scalar.activation(
                        out=vloc_h[:, tt, 0:8, 0:HD],
                        in_=ps1.rearrange("p (h c) -> p h c", c=HD), func=AF.Copy)
                    nc.scalar.activation(
                        out=vloc_h[:, tt, 8:12, 0:HD],
                        in_=ps2.rearrange("p (h c) -> p h c", c=HD), func=AF.Copy)
                nc.sync.dma_start(out=agK, in_=kloc)
                nc.sync.dma_start(out=agV, in_=vloc)
                if os.environ.get("KNOAG", "0") != "1":
                    nc.gpsimd.collective_compute(
                        "AllGather", OP.bypass, replica_groups=G8,
                        ins=[ag_in[:]], outs=[ag_out[:]])
                for m in range(6):
                    ps = psS.tile([128, TOK], F32, tag="psS")
                    for k in range(XKT):
                        nc.tensor.matmul(ps, wattn_sb[:, k, 128 * m:128 * m + 128],
                                         x_ext[:, k, :], start=(k == 0), stop=(k == XKT - 1))
                    nc.scalar.activation(out=qt[:, 2 * m, :], in_=ps[0:64, :], func=AF.Copy)
                    nc.scalar.activation(out=qt[:, 2 * m + 1, :], in_=ps[64:128, :], func=AF.Copy)
                # ---- attention: 2 rounds x 8 k-chunks x 6 heads ----
                for rnd in range(3):
                    yps = []
                    for _yi in range(4):
                        ypt = psY.tile([128, TOK], F32, tag="yps")
                        yps.append(ypt)
                    for kc in range(16):
                        blk8, half = kc // 2, kc % 2
                        kk = kvp.tile([64, 4, 128], BF, tag="kk")
                        nc.sync.dma_start(
                            out=kk, in_=ag_blockK(blk8)[:, 4 * rnd:4 * rnd + 4,
                                                        128 * half:128 * half + 128])
                        vv = kvp.tile([128, 4 * (HD + 1)], BF, tag="vv")
                        nc.sync.dma_start(
                            out=vv, in_=ag_blockV(blk8)[:, half,
                                                        65 * 4 * rnd:65 * 4 * rnd + 260])
                        for hh in range(4):
                            h_ = 4 * rnd + hh
                            ps = psS.tile([128, TOK], F32, tag="psS")
                            nc.tensor.matmul(ps, kk[:, hh, :], qt[:, h_, :],
                                             start=True, stop=True)
                            pt = work.tile([128, TOK], BF, tag="zb")
                            nc.scalar.activation(out=pt, in_=ps, func=AF.Exp,
                                                 scale=1.0 / math.sqrt(HD))
                            nc.vector.tensor_mul(out=pt, in0=pt, in1=msk[:, kc, :])
                            nc.tensor.matmul(yps[hh][0:65, :],
                                             vv[:, 65 * hh:65 * hh + 65], pt,
                                             start=(kc == 0), stop=(kc == 15))
                    for hh in range(4):
                        h_ = 4 * rnd + hh
                        rec = small.tile([1, TOK], BF, tag="rec")
                        with nc.allow_low_precision(reason="softmax denom bf16 ok"):
                            nc.vector.reciprocal(out=rec, in_=yps[hh][64:65, :])
                        bc = psS.tile([64, TOK], F32, tag="psS")
                        nc.tensor.matmul(bc, ones_r[:, 0:64], rec, start=True, stop=True)
                        bcs = work.tile([64, TOK], F32, tag="t2")
                        nc.scalar.activation(out=bcs, in_=bc, func=AF.Copy)
                        nc.vector.tensor_mul(
                            out=ysb[64 * (h_ % 2):64 * (h_ % 2) + 64, h_ // 2, :],
                            in0=yps[hh][0:64, :], in1=bcs)
                # ---- attn_proj ----
                for m in range(DKT):
                    ps = psS.tile([128, TOK], F32, tag="psS")
                    for k in range(DKT):
                        nc.tensor.matmul(ps, wap_sb[:, k, 128 * m:128 * m + 128],
                                         ysb[:, k, :], start=(k == 0), stop=(k == DKT - 1))
                    nc.scalar.activation(out=hsb[:, m, :], in_=ps, func=AF.Identity,
                                         bias=apb_sb[:, m:m + 1])
                # ---- mlp_ln ----
                layernorm([hsb[:, k, :] for k in range(DKT)], DKT, D + TE,
                          l2w_sb, l2b_sb, ltw2[:, blk:blk + 1], ltb2[:, blk:blk + 1],
                          ev, x_ext)
                # ---- c_fc + gelu + mlp_proj (2 halves), z += 0.25*dz ----
                for half in range(3):
                    hf = hfcp.tile([128, 8, TOK], BF, tag="hf")
                    for mm in range(8):
                        m = 8 * half + mm
                        ps = psS.tile([128, TOK], F32, tag="psS")
                        for k in range(XKT):
                            nc.tensor.matmul(ps, wfc_sb[:, k, 128 * m:128 * m + 128],
                                             x_ext[:, k, :], start=(k == 0), stop=(k == XKT - 1))
                        nc.scalar.activation(out=hf[:, mm, :], in_=ps, func=AF.Gelu)
                    for m in range(DKT):
                        ps = psY.tile([128, TOK], F32, tag="yps")
                        for kk_ in range(8):
                            k = 8 * half + kk_
                            nc.tensor.matmul(ps, wmp_sb[:, k, 128 * m:128 * m + 128],
                                             hf[:, kk_, :],
                                             start=(kk_ == 0), stop=(kk_ == 7))
                        if half == 0:
                            nc.scalar.activation(out=hsb[:, m, :], in_=ps, func=AF.Copy)
                        else:
                            tmp = work.tile([128, TOK], F32, tag="t1")
                            nc.scalar.activation(out=tmp, in_=ps, func=AF.Copy)
                            nc.vector.tensor_add(out=hsb[:, m, :], in0=hsb[:, m, :], in1=tmp)
                for m in range(DKT):
                    tmp = work.tile([128, TOK], F32, tag="t1")
                    nc.scalar.activation(out=tmp, in_=hsb[:, m, :],
                                         func=AF.Identity, scale=HSTEP,
                                         bias=mpb_sb[:, m:m + 1])
                    nc.vector.tensor_add(out=z[:, m, :], in0=z[:, m, :], in1=tmp)
                if KDEBUG:
                    for km in range(DKT):
                        dx = work.tile([128, TOK], F32, tag="t1")
                        nc.vector.tensor_copy(out=dx, in_=z[:, km, :])
                        nc.sync.dma_start(out=dbg_z[:, km, :], in_=dx)
                        dy = work.tile([128, TOK], F32, tag="t1")
                        nc.vector.tensor_copy(out=dy, in_=ysb[:, km, :])
                        nc.sync.dma_start(out=dbg_y[:, km, :], in_=dy)
                        dh = work.tile([128, TOK], F32, tag="t1")
                        nc.vector.tensor_copy(out=dh, in_=hsb[:, km, :])
                        nc.sync.dma_start(out=dbg_h[:, km, :], in_=dh)
                    for km in range(XKT):
                        dw = work.tile([128, TOK], F32, tag="t1")
                        nc.vector.tensor_copy(out=dw, in_=x_ext[:, km, :])
                        nc.sync.dma_start(out=dbg_x[:, km, :], in_=dw)
                    for km in range(NH):
                        dq = work.tile([64, TOK], F32, tag="t2")
                        nc.vector.tensor_copy(out=dq, in_=qt[:, km, :])
                        nc.sync.dma_start(out=dbg_q[:, km, :], in_=dq)

        # ---- ln_f -> zf bf16 -> AG all-8 -> lm_head ----
        lfw_sb = wpool.tile([128, DKT], F32, tag="l1w")
        nc.sync.dma_start(out=lfw_sb, in_=lnfw[:, :])
        lfb_sb = wpool.tile([128, DKT], F32, tag="l1b")
        nc.sync.dma_start(out=lfb_sb, in_=lnfb[:, :])
        zf = pers.tile([128, DKT, TOK], BF, tag="zf")
        layernorm([z[:, k, :] for k in range(DKT)], DKT, D,
                  lfw_sb, lfb_sb, None, None, None, zf)
        agfv = agf_in[:].rearrange("(p x) -> p x", p=128)
        nc.sync.dma_start(out=agfv, in_=zf)
        nc.gpsimd.collective_compute(
            "AllGather", OP.bypass, replica_groups=G8,
            ins=[agf_in[:]], outs=[agf_out[:]])

        NC_CHUNK = 256
        nchunks = (VS + NC_CHUNK - 1) // NC_CHUNK
        if os.environ.get("KNULL", "0") == "1":
            nchunks = 1
        for c in range(nchunks):
            cs = min(NC_CHUNK, VS - c * NC_CHUNK)
            wt = hfcp.tile([128, DKT, NC_CHUNK], BF, tag="hf")
            nc.sync.dma_start(out=wt[:, :, 0:cs],
                              in_=wteT[:, :, c * NC_CHUNK:c * NC_CHUNK + cs])
            for tt in range(16):
                r_, hf_ = tt // 2, tt % 2
                zt = kvp.tile([128, DKT, 128], BF, tag="kk")
                o = r_ * 128 * DKT * TOK
                blkv = agf_out[o:o + 128 * DKT * TOK].rearrange(
                    "(p k t) -> p k t", p=128, k=DKT)
                nc.sync.dma_start(out=zt, in_=blkv[:, :, 128 * hf_:128 * hf_ + 128])
                ps = psA.tile([128, NC_CHUNK], F32, tag="psA")
                for k in range(DKT):
                    nc.tensor.matmul(ps[:, 0:cs], zt[:, k, :], wt[:, k, 0:cs],
                                     start=(k == 0), stop=(k == DKT - 1))
                st_ = work.tile([128, NC_CHUNK], F32, tag="t1")
                nc.scalar.activation(out=st_[:, 0:cs], in_=ps[:, 0:cs], func=AF.Copy)
                nc.sync.dma_start(
                    out=logits[128 * tt:128 * tt + 128, c * NC_CHUNK:c * NC_CHUNK + cs],
                    in_=st_[:, 0:cs])

    nc.compile()
    return nc


def _gelu(x):
    from scipy.special import erf
    return 0.5 * x * (1.0 + erf(x / np.sqrt(2.0)))


def _prep(inputs):
    f32 = np.float32
    bf = ml_dtypes.bfloat16
    idx = np.asarray(inputs["idx"]).astype(np.int64)
    wte = np.asarray(inputs["wte"], f32)
    wpe = np.asarray(inputs["wpe"], f32)
    z0 = wte[idx] + wpe[None, :T]                      # [B, T, D]

    tembs = np.zeros((NEV, TE), f32)
    for b in range(NBLK):
        w1 = np.asarray(inputs["time_w1"], f32)[b][0]   # [TE]
        b1 = np.asarray(inputs["time_b1"], f32)[b]
        w2 = np.asarray(inputs["time_w2"], f32)[b]
        b2 = np.asarray(inputs["time_b2"], f32)[b]
        for s in range(NSTEPS):
            t = s * HSTEP
            th = _gelu(t * w1 + b1)
            tembs[b * NSTEPS + s] = th @ w2 + b2
    tcon = np.zeros((1, 2 * NEV), f32)
    tcon[0, :NEV] = tembs.sum(axis=1)
    tcon[0, NEV:] = (tembs ** 2).sum(axis=1)

    def kext(wname, bname, ncols):
        w = np.asarray(inputs[wname], f32)              # [NBLK, 832, ncols]
        b_ = np.asarray(inputs[bname], f32)             # [NBLK, ncols]
        out = np.zeros((NBLK, XKT * 128, ncols), f32)
        out[:, :D + TE] = w
        out[:, D + TE] = b_
        return out.reshape(NBLK, XKT, 128, ncols).transpose(0, 2, 1, 3).astype(bf)

    wattn = kext("c_attn_w", "c_attn_b", 3 * D)
    wfc = kext("c_fc_w", "c_fc_b", 4 * D)
    wap = np.asarray(inputs["attn_proj_w"], f32).reshape(NBLK, DKT, 128, D) \
        .transpose(0, 2, 1, 3).astype(bf)
    apb = np.asarray(inputs["attn_proj_b"], f32).reshape(NBLK, DKT, 128) \
        .transpose(0, 2, 1).astype(f32)
    wmp = np.asarray(inputs["mlp_proj_w"], f32).reshape(NBLK, 4 * DKT, 128, D) \
        .transpose(0, 2, 1, 3).astype(bf)
    mpb = (HSTEP * np.asarray(inputs["mlp_proj_b"], f32)).reshape(NBLK, DKT, 128) \
        .transpose(0, 2, 1).astype(f32)

    def lnz(name):
        a = np.asarray(inputs[name], f32)[:, :D]
        return a.reshape(NBLK, DKT, 128).transpose(0, 2, 1).astype(f32)

    def lnt(name):
        return np.asarray(inputs[name], f32)[:, D:].T.astype(f32).copy()  # [64, NBLK]

    com = dict(
        wattn=wattn, wfc=wfc, wap=wap, apb=apb, wmp=wmp, mpb=mpb,
        ln1w=lnz("attn_ln_w"), ln1b=lnz("attn_ln_b"),
        ln2w=lnz("mlp_ln_w"), ln2b=lnz("mlp_ln_b"),
        ln1tw=lnt("attn_ln_w"), ln1tb=lnt("attn_ln_b"),
        ln2tw=lnt("mlp_ln_w"), ln2tb=lnt("mlp_ln_b"),
        lnfw=np.tile(np.asarray(inputs["ln_f_w"], f32).reshape(DKT, 128).T[:, :], (1, 1)),
        lnfb=np.asarray(inputs["ln_f_b"], f32).reshape(DKT, 128).T.copy(),
        tembs=tembs.T.copy(), tcon=tcon,
    )
    com["lnfw"] = np.asarray(inputs["ln_f_w"], f32).reshape(DKT, 128).T.copy()

    in_maps = []
    for r in range(R):
        b_, q_ = r // 4, r % 4
        zs = z0[b_, TOK * q_:TOK * q_ + TOK, :].T       # [768, 256]
        z0T = zs.reshape(DKT, 128, TOK).transpose(1, 0, 2).astype(f32).copy()
        p = np.arange(128)[:, None, None]
        kc = np.arange(16)[None, :, None]
        q = np.arange(TOK)[None, None, :]
        key_batch = kc // 8
        key_pos = (kc // 2 % 4) * TOK + (kc % 2) * 128 + p
        m = ((key_pos <= TOK * q_ + q) & (key_batch == b_)).astype(bf)
        shard = wte[VS * r:VS * r + VS, :]              # [VS, 768]
        wteT = shard.T.reshape(DKT, 128, VS).transpose(1, 0, 2).astype(bf).copy()
        im = dict(com)
        im.update(z0=z0T, masks=m, wteT=wteT)
        in_maps.append(im)
    return in_maps


def kernel(**inputs):
    if "nc" not in _CACHED:
        _CACHED["nc"] = build_nc()
    nc = _CACHED["nc"]
    in_maps = _prep(inputs)
    trace = os.environ.get("KTRACE", "0") == "1"
    res = run_bass_kernel_spmd(nc, in_maps, core_ids=list(range(R)), trace=trace)
    _CACHED["last_results"] = res
    parts = [res.results[r]["logits"] for r in range(R)]   # each [2048, VS]
    full = np.concatenate(parts, axis=1)                    # [2048, V]
    return full.reshape(B, T, V).astype(np.float32)


def bench(inputs, iters=3):
    """Device-side execution timing: pre-stage inputs on devices, time the
    sharded PJRT call (excludes host prep + H2D of inputs; includes on-device
    zero-init of donated outputs)."""
    import time
    import jax
    import jax.numpy as jnp
    from jax.sharding import Mesh, PartitionSpec, NamedSharding
    from jax.experimental.shard_map import shard_map
    from concourse import bass2jax, mybir as mb
    from concourse.bass2jax import _bass_exec_p, partition_id_tensor

    if "nc" not in _CACHED:
        _CACHED["nc"] = build_nc()
    nc = _CACHED["nc"]
    in_maps = _prep(inputs)
    n_cores = R

    in_names, out_names, out_avals, zero_shapes = [], [], [], []
    partition_name = nc.partition_id_tensor.name if nc.partition_id_tensor else None
    for alloc in nc.m.functions[0].allocations:
        if not isinstance(alloc, mb.MemoryLocationSet):
            continue
        name = alloc.memorylocations[0].name
        if alloc.kind == "ExternalInput":
            if name != partition_name:
                in_names.append(name)
        elif alloc.kind == "ExternalOutput":
            out_names.append(name)
            shape = tuple(alloc.tensor_shape)
            dtype = mb.dt.np(alloc.dtype)
            out_avals.append(jax.core.ShapedArray(shape, dtype))
            zero_shapes.append((shape, dtype))
    n_params = len(in_names)
    n_outs = len(out_avals)
    all_names = list(in_names) + list(out_names)
    if partition_name is not None:
        all_names.append(partition_name)

    def _body(*args):
        operands = list(args)
        if partition_name is not None:
            operands.append(partition_id_tensor())
        outs = _bass_exec_p.bind(
            *operands, out_avals=tuple(out_avals), in_names=tuple(all_names),
            out_names=tuple(out_names), lowering_input_output_aliases=(),
            sim_require_finite=True, sim_require_nnan=True, nc=nc)
        return tuple(outs)

    devices = jax.devices()[:n_cores]
    mesh = Mesh(np.array(devices), ("core",))
    donate = tuple(range(n_params, n_params + n_outs))
    sharded = jax.jit(shard_map(_body, mesh=mesh,
                                in_specs=(PartitionSpec("core"),) * (n_params + n_outs),
                                out_specs=(PartitionSpec("core"),) * n_outs,
                                check_rep=False),
                      donate_argnums=donate, keep_unused=True)
    sh = NamedSharding(mesh, PartitionSpec("core"))
    concat_in = [jax.device_put(
        np.concatenate([np.asarray(in_maps[c][in_names[i]]) for c in range(n_cores)], axis=0), sh)
        for i in range(n_params)]

    def mkzeros():
        return [jnp.zeros((n_cores * s[0], *s[1:]), d, device=sh) for s, d in zero_shapes]

    times = []
    for it in range(iters + 1):
        z_ = mkzeros()
        jax.block_until_ready(z_)
        t0 = time.perf_counter()
        outs = sharded(*concat_in, *z_)
        jax.block_until_ready(outs)
        dt = time.perf_counter() - t0
        if it > 0:
            times.append(dt)
        del outs
    # zeros-creation baseline
    t0 = time.perf_counter()
    z_ = mkzeros(); jax.block_until_ready(z_)
    zt = time.perf_counter() - t0
    return dict(exec_s=min(times), all=times, zeros_s=zt)



# revision 4
# speedup vs baseline: 1.5762x; 1.5762x over previous
"""CocoNODE forward on 8 Trainium2 NeuronCores.

Sharding: token-parallel. Rank r owns 256 tokens: batch r//4, positions
[256*(r%4), 256*(r%4)+256). z kept feature-major (z^T) resident in SBUF f32.
Per dynamics eval: local qkv (all heads, own tokens), AllGather of K/V within
batch groups [[0..3],[4..7]] (G4), causal attention over the 8 own-batch key
chunks, local attn_proj/MLP. Final: ln_f, all-8 AllGather of z_f,
vocab-sharded (50304/8=6288) tied lm_head. Matmuls bf16, PSUM f32,
residual/LN stats f32. attn_ln/mlp_ln affine folded into c_attn/c_fc
weights host-side; LN rsqrt and softmax reciprocal via ln/exp on ScalarE
(one ACT table set across attention+LN; gelu is the only other set).
"""
import os, sys
sys.path.insert(0, '/opt/trn_rl_repo')

import math
import numpy as np
import ml_dtypes
from contextlib import ExitStack

import concourse.bass as bass
import concourse.tile as tile
from concourse import bacc, mybir
from concourse.bass_utils import run_bass_kernel_spmd

AF = mybir.ActivationFunctionType
OP = mybir.AluOpType
BF = mybir.dt.bfloat16
F32 = mybir.dt.float32

B, T, D, NH, TE, V, NBLK, NSTEPS, HSTEP = 2, 1024, 768, 12, 64, 50304, 4, 4, 0.25
HD = D // NH
EPS = 1e-5
R = 8              # ranks
TOK = 256          # tokens per rank
DKT = 6            # 768/128 k-tiles
XKT = 7            # 896/128 k-tiles (832 ext padded)
VS = V // R        # 6288 vocab shard
NEV = NBLK * NSTEPS
KSZ = 128 * DKT * TOK              # K elems per rank in ag block
VSZ = 128 * 2 * (NH * (HD + 1))    # V elems per rank (780 per token-half)
AGSZ = KSZ + VSZ
NC_CHUNK = 512

_CACHED = {}


def build_nc():
    nc = bacc.Bacc("TRN2", target_bir_lowering=False, debug=False, num_devices=R)

    # ---- inputs (per-core DRAM) ----
    z0 = nc.dram_tensor("z0", [128, DKT, TOK], F32, kind="ExternalInput")
    wattn = nc.dram_tensor("wattn", [NBLK, 128, XKT, 3 * D], BF, kind="ExternalInput")
    wap = nc.dram_tensor("wap", [NBLK, 128, DKT, D], BF, kind="ExternalInput")
    apb = nc.dram_tensor("apb", [NBLK, 128, DKT], F32, kind="ExternalInput")
    wfc = nc.dram_tensor("wfc", [NBLK, 128, XKT, 4 * D], BF, kind="ExternalInput")
    wmp = nc.dram_tensor("wmp", [NBLK, 128, 4 * DKT, D], BF, kind="ExternalInput")
    mpb = nc.dram_tensor("mpb", [NBLK, 128, DKT], F32, kind="ExternalInput")
    lnfw = nc.dram_tensor("lnfw", [128, DKT], F32, kind="ExternalInput")
    lnfb = nc.dram_tensor("lnfb", [128, DKT], F32, kind="ExternalInput")
    tembs = nc.dram_tensor("tembs", [64, NEV], F32, kind="ExternalInput")
    tcon = nc.dram_tensor("tcon", [1, 2 * NEV], F32, kind="ExternalInput")
    masks = nc.dram_tensor("masks", [128, 8, TOK], BF, kind="ExternalInput")
    wteT = nc.dram_tensor("wteT", [128, DKT, VS], BF, kind="ExternalInput")
    logits = nc.dram_tensor("logits", [R * TOK, VS], F32, kind="ExternalOutput")
    KDEBUG = os.environ.get("KDEBUG", "0") == "1"
    if KDEBUG:
        dbg_z = nc.dram_tensor("dbg_z", [128, DKT, TOK], F32, kind="ExternalOutput")
        dbg_x = nc.dram_tensor("dbg_x", [128, XKT, TOK], F32, kind="ExternalOutput")
        dbg_q = nc.dram_tensor("dbg_q", [128, DKT, TOK], F32, kind="ExternalOutput")
        dbg_y = nc.dram_tensor("dbg_y", [128, DKT, TOK], F32, kind="ExternalOutput")
        dbg_h = nc.dram_tensor("dbg_h", [128, DKT, TOK], F32, kind="ExternalOutput")

    # ---- internal DRAM (collectives) ----
    ag_in = nc.dram_tensor("ag_in", [AGSZ], BF)
    ag_out = nc.dram_tensor("ag_out", [4 * AGSZ], BF)
    agf_in = nc.dram_tensor("agf_in", [KSZ], BF)
    agf_out = nc.dram_tensor("agf_out", [R * KSZ], BF, addr_space="Shared")
    G4 = [[0, 1, 2, 3], [4, 5, 6, 7]]
    G8 = [[0, 1, 2, 3, 4, 5, 6, 7]]

    agK = ag_in[0:KSZ].rearrange("(p x) -> p x", p=128)            # [128, 1536]
    agV = ag_in[KSZ:AGSZ].rearrange("(p x) -> p x", p=128)         # [128, 1560]

    with ExitStack() as ctx:
        tc = ctx.enter_context(tile.TileContext(nc))
        pers = ctx.enter_context(tc.tile_pool(name="pers", bufs=1))
        work = ctx.enter_context(tc.tile_pool(name="work", bufs=2))
        small = ctx.enter_context(tc.tile_pool(name="small", bufs=2))
        ptp = ctx.enter_context(tc.tile_pool(name="ptp", bufs=2))
        # PSUM: 8 banks total = psBig 2x2 + psP 2x1 + psA 2x1
        psBig = ctx.enter_context(tc.tile_pool(name="psBig", bufs=2, space="PSUM"))
        psP = ctx.enter_context(tc.tile_pool(name="psP", bufs=2, space="PSUM"))
        psA = ctx.enter_context(tc.tile_pool(name="psA", bufs=2, space="PSUM"))

        z = pers.tile([128, DKT, TOK], F32, tag="z")
        nc.sync.dma_start(out=z, in_=z0[:, :, :])
        msk = pers.tile([128, 8, TOK], BF, tag="msk")
        nc.sync.dma_start(out=msk, in_=masks[:, :, :])
        te_sb = pers.tile([64, NEV], F32, tag="te")
        nc.sync.dma_start(out=te_sb, in_=tembs[:, :])
        tc_sb = pers.tile([1, 2 * NEV], F32, tag="tc")
        nc.sync.dma_start(out=tc_sb, in_=tcon[:, :])

        ones_c = pers.tile([128, 1], BF, tag="ones_c")   # stats lhsT [128,1]
        nc.vector.memset(ones_c, 1.0)
        ones_r = pers.tile([1, 128], BF, tag="ones_r")   # bcast lhsT [1,<=128]
        nc.vector.memset(ones_r, 1.0)
        eps_sb = pers.tile([1, 1], F32, tag="eps")
        nc.vector.memset(eps_sb, EPS)

        x_ext = pers.tile([128, XKT, TOK], BF, tag="x_ext")
        nc.vector.memset(x_ext[64:128, XKT - 1, :], 0.0)
        nc.vector.memset(x_ext[64:65, XKT - 1, :], 1.0)

        qt2 = pers.tile([128, DKT, TOK], BF, tag="qt2")
        kloc2 = pers.tile([128, DKT, TOK], BF, tag="kloc2")
        vloc = pers.tile([128, 2, NH * (HD + 1)], BF, tag="vloc")
        vloc_h = vloc.rearrange("p t (h c) -> p t h c", h=NH)
        nc.vector.memset(vloc_h[:, :, :, HD:HD + 1], 1.0)
        kall = pers.tile([128, 4, DKT, TOK], BF, tag="kall")
        vall = pers.tile([128, 4, 2, NH * (HD + 1)], BF, tag="vall")
        yall = pers.tile([65, NH, TOK], BF, tag="yall")
        dall = pers.tile([1, 6, TOK], BF, tag="dall")
        recall = pers.tile([1, 6, TOK], BF, tag="recall")
        ysb = pers.tile([128, DKT, TOK], BF, tag="ysb")
        hsb = pers.tile([128, DKT, TOK], F32, tag="hsb")
        mu_sb = pers.tile([128, TOK], F32, tag="mu_sb")
        rs_sb = pers.tile([128, TOK], F32, tag="rs_sb")

        def layernorm(src_tiles, nkt, divisor, t_ev, dst, wt=None, bt=None):
            """(src - mu) * rsigma -> dst (bf16). t_ev not None: also write temb
            part to dst tile XKT-1 rows 0:64. wt/bt: extra affine (ln_f only)."""
            sums = psBig.tile([1, TOK], F32, tag="qk")
            sumsq = psBig.tile([1, TOK], F32, tag="qk")
            for k in range(nkt):
                zb = work.tile([128, TOK], BF, tag="zb")
                nc.vector.tensor_copy(out=zb, in_=src_tiles[k])
                zq = work.tile([128, TOK], BF, tag="zq")
                nc.vector.tensor_mul(out=zq, in0=zb, in1=zb)
                nc.tensor.matmul(sums, ones_c, zb, start=(k == 0), stop=(k == nkt - 1))
                nc.tensor.matmul(sumsq, ones_c, zq, start=(k == 0), stop=(k == nkt - 1))
            mu = small.tile([1, TOK], F32, tag="mu")
            e2 = small.tile([1, TOK], F32, tag="e2")
            if t_ev is not None:
                nc.vector.tensor_scalar(out=mu, in0=sums, scalar1=tc_sb[0:1, t_ev:t_ev + 1],
                                        scalar2=1.0 / divisor, op0=OP.add, op1=OP.mult)
                nc.vector.tensor_scalar(out=e2, in0=sumsq,
                                        scalar1=tc_sb[0:1, NEV + t_ev:NEV + t_ev + 1],
                                        scalar2=1.0 / divisor, op0=OP.add, op1=OP.mult)
            else:
                nc.vector.tensor_scalar_mul(out=mu, in0=sums, scalar1=1.0 / divisor)
                nc.vector.tensor_scalar_mul(out=e2, in0=sumsq, scalar1=1.0 / divisor)
            mu2 = small.tile([1, TOK], F32, tag="mu2")
            nc.vector.tensor_mul(out=mu2, in0=mu, in1=mu)
            var = small.tile([1, TOK], F32, tag="var")
            nc.vector.tensor_sub(out=var, in0=e2, in1=mu2)
            # rsigma = exp(-0.5*ln(var+eps)); ln/exp share one ACT table set
            lv = small.tile([1, TOK], F32, tag="lv")
            nc.scalar.activation(out=lv, in_=var, func=AF.Ln, bias=eps_sb, scale=1.0)
            rst = small.tile([1, TOK], F32, tag="rst")
            nc.scalar.activation(out=rst, in_=lv, func=AF.Exp, scale=-0.5)
            mu_bf = small.tile([1, TOK], BF, tag="mu_bf")
            nc.vector.tensor_copy(out=mu_bf, in_=mu)
            rs_bf = small.tile([1, TOK], BF, tag="rs_bf")
            nc.vector.tensor_copy(out=rs_bf, in_=rst)
            mu_ps = psBig.tile([128, TOK], F32, tag="qk")
            nc.tensor.matmul(mu_ps, ones_r, mu_bf, start=True, stop=True)
            nc.vector.tensor_copy(out=mu_sb, in_=mu_ps)
            rs_ps = psBig.tile([128, TOK], F32, tag="qk")
            nc.tensor.matmul(rs_ps, ones_r, rs_bf, start=True, stop=True)
            nc.scalar.activation(out=rs_sb, in_=rs_ps, func=AF.Copy)
            for k in range(nkt):
                t1 = work.tile([128, TOK], F32, tag="t1")
                nc.vector.tensor_sub(out=t1, in0=src_tiles[k], in1=mu_sb)
                if wt is None:
                    nc.vector.tensor_mul(out=dst[:, k, :], in0=t1, in1=rs_sb)
                else:
                    nc.vector.tensor_mul(out=t1, in0=t1, in1=rs_sb)
                    nc.vector.tensor_scalar(out=dst[:, k, :], in0=t1,
                                            scalar1=wt[:, k:k + 1], scalar2=bt[:, k:k + 1],
                                            op0=OP.mult, op1=OP.add)
            if t_ev is not None:
                t2 = work.tile([64, TOK], F32, tag="t2")
                nc.vector.tensor_scalar(out=t2, in0=mu_sb[0:64, :], scalar1=-1.0,
                                        scalar2=te_sb[:, t_ev:t_ev + 1], op0=OP.mult, op1=OP.add)
                nc.vector.tensor_mul(out=dst[0:64, XKT - 1, :], in0=t2, in1=rs_sb[0:64, :])

        NBLK_RUN = 1 if KDEBUG else NBLK
        NSTEPS_RUN = 1 if KDEBUG else NSTEPS
        if os.environ.get("KNULL", "0") == "1":
            NBLK_RUN = 0
        KREPEAT = int(os.environ.get("KREPEAT", "1"))

        with tc.tile_pool(name="wres", bufs=1) as wres, \
             tc.tile_pool(name="wstr", bufs=2) as wstr:
            wap_sb = wres.tile([128, DKT, D], BF, tag="wap")
            apb_sb = wres.tile([128, DKT], F32, tag="apb")
            mpb_sb = wres.tile([128, DKT], F32, tag="mpb")

            for blk in [b % NBLK for b in range(NBLK_RUN * KREPEAT)]:
                nc.sync.dma_start(out=wap_sb, in_=wap[blk])
                nc.sync.dma_start(out=apb_sb, in_=apb[blk])
                nc.sync.dma_start(out=mpb_sb, in_=mpb[blk])
                for st in range(NSTEPS_RUN):
                    ev = blk * NSTEPS + st
                    # ---- attn_ln (affine folded into wattn) ----
                    layernorm([z[:, k, :] for k in range(DKT)], DKT, D + TE, ev, x_ext)
                    # ---- qkv: K first (feeds AG), then V, then Q ----
                    wat_k = wstr.tile([128, XKT, D], BF, tag="wat")
                    nc.sync.dma_start(out=wat_k, in_=wattn[blk, :, :, D:2 * D])
                    for m in range(6):
                        ps = psP.tile([128, TOK], F32, tag="ps")
                        for k in range(XKT):
                            nc.tensor.matmul(ps, wat_k[:, k, 128 * m:128 * m + 128],
                                             x_ext[:, k, :], start=(k == 0), stop=(k == XKT - 1))
                        nc.scalar.activation(out=kloc2[:, m, :], in_=ps, func=AF.Copy)
                    nc.sync.dma_start(out=agK, in_=kloc2)
                    wat_v = wstr.tile([128, XKT, D], BF, tag="wat")
                    nc.sync.dma_start(out=wat_v, in_=wattn[blk, :, :, 2 * D:3 * D])
                    for tt in range(2):
                        ps1 = psP.tile([128, 512], F32, tag="ps")
                        ps2 = psP.tile([128, TOK], F32, tag="ps")
                        for k in range(XKT):
                            nc.tensor.matmul(ps1, x_ext[:, k, 128 * tt:128 * tt + 128],
                                             wat_v[:, k, 0:512],
                                             start=(k == 0), stop=(k == XKT - 1))
                        for k in range(XKT):
                            nc.tensor.matmul(ps2, x_ext[:, k, 128 * tt:128 * tt + 128],
                                             wat_v[:, k, 512:768],
                                             start=(k == 0), stop=(k == XKT - 1))
                        nc.scalar.activation(
                            out=vloc_h[:, tt, 0:8, 0:HD],
                            in_=ps1.rearrange("p (h c) -> p h c", c=HD), func=AF.Copy)
                        nc.scalar.activation(
                            out=vloc_h[:, tt, 8:12, 0:HD],
                            in_=ps2.rearrange("p (h c) -> p h c", c=HD), func=AF.Copy)
                    nc.sync.dma_start(out=agV, in_=vloc)
                    if os.environ.get("KNOAG", "0") != "1":
                        nc.gpsimd.collective_compute(
                            "AllGather", OP.bypass, replica_groups=G4,
                            ins=[ag_in[:]], outs=[ag_out[:]])
                    wat_q = wstr.tile([128, XKT, D], BF, tag="wat")
                    nc.sync.dma_start(out=wat_q, in_=wattn[blk, :, :, 0:D])
                    for m in range(6):
                        ps = psP.tile([128, TOK], F32, tag="ps")
                        for k in range(XKT):
                            nc.tensor.matmul(ps, wat_q[:, k, 128 * m:128 * m + 128],
                                             x_ext[:, k, :], start=(k == 0), stop=(k == XKT - 1))
                        nc.vector.tensor_copy(out=qt2[:, m, :], in_=ps)
                    # ---- pull gathered K/V into SBUF (8 big DMAs) ----
                    for b in range(4):
                        o = b * AGSZ
                        blkK = ag_out[o:o + KSZ].rearrange("(p x) -> p x", p=128)
                        nc.sync.dma_start(out=kall[:, b, :, :], in_=blkK)
                        blkV = ag_out[o + KSZ:o + AGSZ].rearrange("(p x) -> p x", p=128)
                        nc.sync.dma_start(out=vall[:, b, :, :], in_=blkV)
                    # ---- attention: 12 heads, 8 causal key chunks ----
                    for h in range(NH):
                        par, m = h % 2, h // 2
                        pt = ptp.tile([128, 8, TOK], BF, tag="pt")
                        for half in range(2):
                            reg = psBig.tile([128, 4, TOK], F32, tag="qk")
                            for kc4 in range(4):
                                kc = 4 * half + kc4
                                b, tt = kc // 2, kc % 2
                                nc.tensor.matmul(
                                    reg[:, kc4, :],
                                    kall[64 * par:64 * par + 64, b, m,
                                         128 * tt:128 * tt + 128],
                                    qt2[64 * par:64 * par + 64, m, :],
                                    start=True, stop=True)
                            nc.scalar.activation(
                                out=pt[:, 4 * half:4 * half + 4, :],
                                in_=reg, func=AF.Exp, scale=1.0 / math.sqrt(HD))
                        nc.vector.tensor_mul(out=pt, in0=pt, in1=msk)
                        av = psA.tile([65, TOK], F32, tag="av")
                        for kc in range(8):
                            b, tt = kc // 2, kc % 2
                            nc.tensor.matmul(av, vall[:, b, tt, 65 * h:65 * h + 65],
                                             pt[:, kc, :], start=(kc == 0), stop=(kc == 7))
                        nc.vector.tensor_copy(out=yall[:, h, :], in_=av)
                        nc.vector.tensor_copy(out=dall[:, h % 6, :], in_=av[64:65, :])
                        if h % 6 == 5:
                            # reciprocal of 6 heads' denominators: 1/d = exp(-ln d)
                            lnd = small.tile([1, 6 * TOK], F32, tag="lnd", bufs=1)
                            nc.scalar.activation(out=lnd, in_=dall.rearrange("p a b -> p (a b)"),
                                                 func=AF.Ln)
                            nc.scalar.activation(out=recall.rearrange("p a b -> p (a b)"),
                                                 in_=lnd, func=AF.Exp, scale=-1.0)
                            for hh in range(h - 5, h + 1):
                                bc = psP.tile([64, TOK], F32, tag="ps")
                                nc.tensor.matmul(bc, ones_r[:, 0:64], recall[:, hh % 6, :],
                                                 start=True, stop=True)
                                bcs = work.tile([64, TOK], BF, tag="bcs")
                                nc.vector.tensor_copy(out=bcs, in_=bc)
                                nc.vector.tensor_mul(
                                    out=ysb[64 * (hh % 2):64 * (hh % 2) + 64, hh // 2, :],
                                    in0=yall[0:64, hh, :], in1=bcs)
                    # ---- attn_proj (bias via DVE) ----
                    for m in range(DKT):
                        ps = psP.tile([128, TOK], F32, tag="ps")
                        for k in range(DKT):
                            nc.tensor.matmul(ps, wap_sb[:, k, 128 * m:128 * m + 128],
                                             ysb[:, k, :], start=(k == 0), stop=(k == DKT - 1))
                        nc.vector.tensor_scalar_add(out=hsb[:, m, :], in0=ps,
                                                    scalar1=apb_sb[:, m:m + 1])
                    # ---- mlp_ln ----
                    layernorm([hsb[:, k, :] for k in range(DKT)], DKT, D + TE, ev, x_ext)
                    # ---- c_fc + gelu + mlp_proj; z += dz (0.25 pre-folded) ----
                    for third in range(3):
                        wfc_t = wstr.tile([128, XKT, 1024], BF, tag="wfc")
                        nc.sync.dma_start(out=wfc_t,
                                          in_=wfc[blk, :, :, 1024 * third:1024 * third + 1024])
                        wmp_t = wstr.tile([128, 8, D], BF, tag="wmp")
                        nc.sync.dma_start(out=wmp_t,
                                          in_=wmp[blk, :, 8 * third:8 * third + 8, :])
                        hf = ptp.tile([128, 8, TOK], BF, tag="pt")
                        for mm in range(4):
                            gps = psP.tile([128, 512], F32, tag="ps")
                            for two in range(2):
                                mloc = 2 * mm + two
                                for k in range(XKT):
                                    nc.tensor.matmul(gps[:, 256 * two:256 * two + 256],
                                                     wfc_t[:, k, 128 * mloc:128 * mloc + 128],
                                                     x_ext[:, k, :],
                                                     start=(k == 0), stop=(k == XKT - 1))
                            nc.scalar.activation(
                                out=hf[:, 2 * mm:2 * mm + 2, :],
                                in_=gps.rearrange("p (a b) -> p a b", a=2), func=AF.Gelu)
                        for m in range(DKT):
                            ps = psP.tile([128, TOK], F32, tag="ps")
                            for kk_ in range(8):
                                nc.tensor.matmul(ps, wmp_t[:, kk_, 128 * m:128 * m + 128],
                                                 hf[:, kk_, :],
                                                 start=(kk_ == 0), stop=(kk_ == 7))
                            if third == 0:
                                nc.vector.tensor_copy(out=hsb[:, m, :], in_=ps)
                            else:
                                nc.vector.tensor_add(out=hsb[:, m, :], in0=ps, in1=hsb[:, m, :])
                    for m in range(DKT):
                        tmp = work.tile([128, TOK], F32, tag="t1")
                        nc.vector.tensor_scalar_add(out=tmp, in0=hsb[:, m, :],
                                                    scalar1=mpb_sb[:, m:m + 1])
                        nc.vector.tensor_add(out=z[:, m, :], in0=z[:, m, :], in1=tmp)
                    if KDEBUG:
                        for km in range(DKT):
                            for src, dstd in ((z, dbg_z), (ysb, dbg_y), (hsb, dbg_h),
                                              (qt2, dbg_q)):
                                dx = work.tile([128, TOK], F32, tag="t1")
                                nc.vector.tensor_copy(out=dx, in_=src[:, km, :])
                                nc.sync.dma_start(out=dstd[:, km, :], in_=dx)
                        for km in range(XKT):
                            dw = work.tile([128, TOK], F32, tag="t1")
                            nc.vector.tensor_copy(out=dw, in_=x_ext[:, km, :])
                            nc.sync.dma_start(out=dbg_x[:, km, :], in_=dw)

        # ---- ln_f -> zf bf16 -> AG all-8 -> lm_head ----
        with tc.tile_pool(name="lmw", bufs=1) as lmw, \
             tc.tile_pool(name="lmwt", bufs=2) as lmwt:
            lfw_sb = lmw.tile([128, DKT], F32, tag="lfw")
            nc.sync.dma_start(out=lfw_sb, in_=lnfw[:, :])
            lfb_sb = lmw.tile([128, DKT], F32, tag="lfb")
            nc.sync.dma_start(out=lfb_sb, in_=lnfb[:, :])
            zf = ysb  # reuse [128, DKT, TOK] bf16
            layernorm([z[:, k, :] for k in range(DKT)], DKT, D, None, zf,
                      wt=lfw_sb, bt=lfb_sb)
            agfv = agf_in[:].rearrange("(p x) -> p x", p=128)
            nc.sync.dma_start(out=agfv, in_=zf)
            nc.gpsimd.collective_compute(
                "AllGather", OP.bypass, replica_groups=G8,
                ins=[agf_in[:]], outs=[agf_out[:]])
            zsb = lmw.tile([128, R, DKT, TOK], BF, tag="zsb")
            for r_ in range(R):
                o = r_ * KSZ
                blkz = agf_out[o:o + KSZ].rearrange("(p x) -> p x", p=128)
                nc.sync.dma_start(out=zsb[:, r_, :, :], in_=blkz)

            nchunks = (VS + NC_CHUNK - 1) // NC_CHUNK
            if os.environ.get("KNULL", "0") == "1":
                nchunks = 1
            for c in range(nchunks):
                cs = min(NC_CHUNK, VS - c * NC_CHUNK)
                wt = lmwt.tile([128, DKT, NC_CHUNK], BF, tag="wt")
                nc.sync.dma_start(out=wt[:, :, 0:cs],
                                  in_=wteT[:, :, c * NC_CHUNK:c * NC_CHUNK + cs])
                for tt in range(16):
                    r_, hf_ = tt // 2, tt % 2
                    ps = psP.tile([128, NC_CHUNK], F32, tag="ps")
                    for k in range(DKT):
                        nc.tensor.matmul(ps[:, 0:cs],
                                         zsb[:, r_, k, 128 * hf_:128 * hf_ + 128],
                                         wt[:, k, 0:cs], start=(k == 0), stop=(k == DKT - 1))
                    st_ = work.tile([128, NC_CHUNK], F32, tag="st")
                    if tt % 2 == 0:
                        nc.vector.tensor_copy(out=st_[:, 0:cs], in_=ps[:, 0:cs])
                    else:
                        nc.scalar.activation(out=st_[:, 0:cs], in_=ps[:, 0:cs], func=AF.Copy)
                    nc.sync.dma_start(
                        out=logits[128 * tt:128 * tt + 128, c * NC_CHUNK:c * NC_CHUNK + cs],
                        in_=st_[:, 0:cs])

    nc.compile()
    return nc


def _gelu(x):
    from scipy.special import erf
    return 0.5 * x * (1.0 + erf(x / np.sqrt(2.0)))


def _prep(inputs):
    f32 = np.float32
    bf = ml_dtypes.bfloat16
    idx = np.asarray(inputs["idx"]).astype(np.int64)
    wte = np.asarray(inputs["wte"], f32)
    wpe = np.asarray(inputs["wpe"], f32)
    z0 = wte[idx] + wpe[None, :T]                      # [B, T, D]

    tembs = np.zeros((NEV, TE), f32)
    for b in range(NBLK):
        w1 = np.asarray(inputs["time_w1"], f32)[b][0]   # [TE]
        b1 = np.asarray(inputs["time_b1"], f32)[b]
        w2 = np.asarray(inputs["time_w2"], f32)[b]
        b2 = np.asarray(inputs["time_b2"], f32)[b]
        for s in range(NSTEPS):
            t = s * HSTEP
            th = _gelu(t * w1 + b1)
            tembs[b * NSTEPS + s] = th @ w2 + b2
    tcon = np.zeros((1, 2 * NEV), f32)
    tcon[0, :NEV] = tembs.sum(axis=1)
    tcon[0, NEV:] = (tembs ** 2).sum(axis=1)

    def kext(wname, bname, lnw_name, lnb_name, ncols, scale=1.0):
        w = np.asarray(inputs[wname], f32)              # [NBLK, 832, ncols]
        b_ = np.asarray(inputs[bname], f32)             # [NBLK, ncols]
        lnw = np.asarray(inputs[lnw_name], f32)         # [NBLK, 832]
        lnb = np.asarray(inputs[lnb_name], f32)
        w_eff = w * lnw[:, :, None]
        b_eff = b_ + np.einsum('nfo,nf->no', w, lnb)
        out = np.zeros((NBLK, XKT * 128, ncols), f32)
        out[:, :D + TE] = w_eff * scale
        out[:, D + TE] = b_eff * scale
        return out.reshape(NBLK, XKT, 128, ncols).transpose(0, 2, 1, 3).astype(bf)

    wattn = kext("c_attn_w", "c_attn_b", "attn_ln_w", "attn_ln_b", 3 * D)
    wfc = kext("c_fc_w", "c_fc_b", "mlp_ln_w", "mlp_ln_b", 4 * D)
    wap = np.asarray(inputs["attn_proj_w"], f32).reshape(NBLK, DKT, 128, D) \
        .transpose(0, 2, 1, 3).astype(bf)
    apb = np.asarray(inputs["attn_proj_b"], f32).reshape(NBLK, DKT, 128) \
        .transpose(0, 2, 1).astype(f32)
    wmp = (HSTEP * np.asarray(inputs["mlp_proj_w"], f32)).reshape(NBLK, 4 * DKT, 128, D) \
        .transpose(0, 2, 1, 3).astype(bf)
    mpb = (HSTEP * np.asarray(inputs["mlp_proj_b"], f32)).reshape(NBLK, DKT, 128) \
        .transpose(0, 2, 1).astype(f32)

    com = dict(
        wattn=wattn, wfc=wfc, wap=wap, apb=apb, wmp=wmp, mpb=mpb,
        lnfw=np.asarray(inputs["ln_f_w"], f32).reshape(DKT, 128).T.copy(),
        lnfb=np.asarray(inputs["ln_f_b"], f32).reshape(DKT, 128).T.copy(),
        tembs=tembs.T.copy(), tcon=tcon,
    )

    in_maps = []
    for r in range(R):
        b_, q_ = r // 4, r % 4
        zs = z0[b_, TOK * q_:TOK * q_ + TOK, :].T       # [768, 256]
        z0T = zs.reshape(DKT, 128, TOK).transpose(1, 0, 2).astype(f32).copy()
        p = np.arange(128)[:, None, None]
        kc = np.arange(8)[None, :, None]
        q = np.arange(TOK)[None, None, :]
        key_pos = (kc // 2) * TOK + (kc % 2) * 128 + p
        m = (key_pos <= TOK * q_ + q).astype(bf)
        shard = wte[VS * r:VS * r + VS, :]              # [VS, 768]
        wteT = shard.T.reshape(DKT, 128, VS).transpose(1, 0, 2).astype(bf).copy()
        im = dict(com)
        im.update(z0=z0T, masks=m, wteT=wteT)
        in_maps.append(im)
    return in_maps


def kernel(**inputs):
    if "nc" not in _CACHED:
        _CACHED["nc"] = build_nc()
    nc = _CACHED["nc"]
    in_maps = _prep(inputs)
    trace = os.environ.get("KTRACE", "0") == "1"
    res = run_bass_kernel_spmd(nc, in_maps, core_ids=list(range(R)), trace=trace)
    _CACHED["last_results"] = res
    parts = [res.results[r]["logits"] for r in range(R)]   # each [2048, VS]
    full = np.concatenate(parts, axis=1)                    # [2048, V]
    return full.reshape(B, T, V).astype(np.float32)
